# revision 10
# baseline (speedup 1.0000x reference)
"""Trainium2 Bass kernel for nn_ANPM_5583457485031 (attention-pooled graph-pair similarity).

Sharding: pure data-parallel over the B=8 graph pairs (one pair per NeuronCore).
Per core, each graph's (100000, 128) node matrix is processed in 3 passes
(mean -> attention round 1 -> attention round 2), each a streamed reduction.
Key simplification: with K=1 the L1-normalize makes every attention score
exactly +-1, so per-node attention weights take only two values
(U*sigmoid(+-1)); each round reduces to per-node dot products (TensorE via
per-tile PE transpose) plus attention-weighted column sums (TensorE matmuls
accumulated in PSUM). The tiny NTN + projection head runs on host.
"""

import sys

import numpy as np

sys.path.insert(0, "/opt/trn_rl_repo")

import concourse.bass as bass
import concourse.bacc as bacc
import concourse.mybir as mybir
from concourse.tile import TileContext
from concourse.bass_utils import run_bass_kernel_spmd

F32 = mybir.dt.float32
B, N, D = 8, 100000, 128
NH = 2                       # attention heads
CH = 2048                    # nodes per chunk
NT = CH // 128               # 16 tiles of 128 nodes per chunk
FULL = N // CH               # 48 full chunks
NCHUNK = FULL + 1            # 49 (last is the padded tail)
TAILN = N - FULL * CH        # 1696 = 13*128 + 32
EPS = 1e-12

_CACHED = {}


def _build_nc():
    nc = bacc.Bacc()
    xs = [
        nc.declare_dram_parameter("x1", [N, D], F32, isOutput=False),
        nc.declare_dram_parameter("x2", [N, D], F32, isOutput=False),
    ]
    wn_ext = nc.declare_dram_parameter("wn", [D, NH * D], F32, isOutput=False)
    wtt_ext = nc.declare_dram_parameter("wtt", [D, NH * D], F32, isOutput=False)
    vat_ext = nc.declare_dram_parameter("vat", [D, NH], F32, isOutput=False)
    vbt_ext = nc.declare_dram_parameter("vbt", [D, NH], F32, isOutput=False)
    negb_ext = nc.declare_dram_parameter("negb", [D, NH], F32, isOutput=False)
    lo32_ext = nc.declare_dram_parameter("lo32", [D, NT * NH], F32, isOutput=False)
    hm32_ext = nc.declare_dram_parameter("hm32", [D, NT * NH], F32, isOutput=False)
    id_ext = nc.declare_dram_parameter("ident", [D, D], F32, isOutput=False)
    out_ext = nc.declare_dram_parameter("out", [2, NH, D], F32, isOutput=True)

    TT = nc.vector.tensor_tensor
    OP = mybir.AluOpType

    with TileContext(nc) as tc:
        with (
            tc.tile_pool(name="xin", bufs=3) as p_x,
            tc.tile_pool(name="xts", bufs=3) as p_xts,
            tc.tile_pool(name="small", bufs=2) as p_sm,
            tc.tile_pool(name="attb", bufs=2) as p_att,
            tc.tile_pool(name="consts", bufs=1) as p_c,
            tc.tile_pool(name="ps_xt", bufs=2, space="PSUM") as pp_xt,
            tc.tile_pool(name="ps_d", bufs=2, space="PSUM") as pp_d,
            tc.tile_pool(name="ps_acc", bufs=1, space="PSUM") as pp_acc,
            tc.tile_pool(name="ps_sm", bufs=3, space="PSUM") as pp_sm,
        ):
            # ---- constants into SBUF ----
            wn_sb = p_c.tile([D, NH * D], F32, tag="wn")
            nc.sync.dma_start(out=wn_sb[:], in_=wn_ext[:, :])
            wtt_sb = p_c.tile([D, NH * D], F32, tag="wtt")
            nc.sync.dma_start(out=wtt_sb[:], in_=wtt_ext[:, :])
            vat_sb = p_c.tile([D, NH], F32, tag="vat")
            nc.sync.dma_start(out=vat_sb[:], in_=vat_ext[:, :])
            vbt_sb = p_c.tile([D, NH], F32, tag="vbt")
            nc.sync.dma_start(out=vbt_sb[:], in_=vbt_ext[:, :])
            negb_sb = p_c.tile([D, NH], F32, tag="negb")
            nc.sync.dma_start(out=negb_sb[:], in_=negb_ext[:, :])
            lo32_sb = p_c.tile([D, NT * NH], F32, tag="lo32")
            nc.sync.dma_start(out=lo32_sb[:], in_=lo32_ext[:, :])
            hm32_sb = p_c.tile([D, NT * NH], F32, tag="hm32")
            nc.sync.dma_start(out=hm32_sb[:], in_=hm32_ext[:, :])
            ident_sb = p_c.tile([D, D], F32, tag="ident")
            nc.sync.dma_start(out=ident_sb[:], in_=id_ext[:, :])
            ones_col = p_c.tile([D, 1], F32, tag="ones")
            nc.vector.memset(ones_col[:], 1.0)
            mones_row = p_c.tile([1, D], F32, tag="mones")
            nc.vector.memset(mones_row[:], -1.0)

            def load_chunk(g, c, xt):
                if c < FULL:
                    src = xs[g][c * CH:(c + 1) * CH, :].rearrange(
                        "(cb p) d -> p cb d", p=128)
                    nc.sync.dma_start(
                        out=xt[:].rearrange("p (cb d) -> p cb d", d=D), in_=src)
                else:
                    nc.gpsimd.memset(xt[:], 0.0)
                    n0 = FULL * CH
                    nfull = (TAILN // 128) * 128  # 1664
                    src1 = xs[g][n0:n0 + nfull, :].rearrange(
                        "(cb p) d -> p cb d", p=128)
                    nc.sync.dma_start(
                        out=xt[:, 0:nfull].rearrange("p (cb d) -> p cb d", d=D),
                        in_=src1)
                    rem = TAILN - nfull  # 32
                    src2 = xs[g][n0 + nfull:N, :]
                    nc.sync.dma_start(
                        out=xt[0:rem, nfull:nfull + 128], in_=src2)

            def att_params(scol_sb, colmap):
                """From pooled column(s) (128, >=1) compute C (128,2) and -beta
                broadcast (128, NT*NH)."""
                c_ps = pp_sm.tile([D, NH], F32, tag="spsum")
                beta_ps = pp_sm.tile([1, NH], F32, tag="spsum")
                for i in range(NH):
                    h_ps = pp_sm.tile([D, 1], F32, tag="spsum")
                    nc.tensor.matmul(
                        h_ps[:], wn_sb[:, i * D:(i + 1) * D],
                        scol_sb[:, colmap[i]:colmap[i] + 1],
                        start=True, stop=True)
                    h_sb = p_sm.tile([D, 1], F32, tag="h_sb")
                    nc.scalar.activation(
                        h_sb[:], h_ps[:], mybir.ActivationFunctionType.Tanh)
                    nc.tensor.matmul(
                        c_ps[:, i:i + 1], wtt_sb[:, i * D:(i + 1) * D], h_sb[:],
                        start=True, stop=True)
                    nc.tensor.matmul(
                        beta_ps[:, i:i + 1], h_sb[:], vbt_sb[:, i:i + 1],
                        start=True, stop=True)
                C_sb = p_sm.tile([D, NH], F32, tag="C_sb")
                TT(C_sb[:], c_ps[:], vat_sb[:], OP.add)
                beta_sb = p_sm.tile([1, NH], F32, tag="beta_sb")
                nc.vector.tensor_copy(beta_sb[:], beta_ps[:])
                nb_ps = pp_sm.tile([D, NH], F32, tag="spsum")
                nc.tensor.matmul(nb_ps[:], mones_row[:], beta_sb[:],
                                 start=True, stop=True)
                nb_sb = p_sm.tile([D, NH], F32, tag="nb_sb")
                TT(nb_sb[:], nb_ps[:], negb_sb[:], OP.add)
                nb32 = p_sm.tile([D, NT * NH], F32, tag="nb32")
                nc.vector.tensor_copy(
                    nb32[:].rearrange("p (t h) -> p t h", h=NH),
                    nb_sb[:, None, :].to_broadcast((D, NT, NH)))
                return C_sb, nb32

            def row_to_cols(row_sb, r):
                tr_ps = pp_sm.tile([D, NH], F32, tag="spsum")
                nc.tensor.transpose(
                    tr_ps[:, 0:r], row_sb[:], ident_sb[0:r, 0:r])
                cols = p_sm.tile([D, NH], F32, tag="scols")
                nc.vector.tensor_copy(cols[:, 0:r], tr_ps[:, 0:r])
                return cols

            def dots(xt, C_sb, d_ps):
                """Per-node dot products for one chunk: d_ps (128, NT*NH)."""
                for q in range(NT // 4):
                    xt_ps = pp_xt.tile([128, 512], F32, tag="xtps")
                    for t4 in range(4):
                        t = q * 4 + t4
                        nc.tensor.transpose(
                            xt_ps[:, t4 * 128:(t4 + 1) * 128],
                            xt[:, t * 128:(t + 1) * 128], ident_sb[:])
                    xt_sb = p_xts.tile([128, 512], F32, tag="xts")
                    if q % 2 == 0:
                        nc.vector.tensor_copy(xt_sb[:], xt_ps[:])
                    else:
                        nc.scalar.copy(xt_sb[:], xt_ps[:])
                    for t4 in range(4):
                        t = q * 4 + t4
                        nc.tensor.matmul(
                            d_ps[:, t * NH:(t + 1) * NH],
                            xt_sb[:, t4 * 128:(t4 + 1) * 128], C_sb[:],
                            start=True, stop=True)

            for g in range(2):
                # ---------- pass A: column sums ----------
                s0_ps = pp_acc.tile([1, 512], F32, tag="sacc")
                for c in range(NCHUNK):
                    xt = p_x.tile([128, CH], F32, tag="xt")
                    load_chunk(g, c, xt)
                    for j in range(CH // 512):
                        nc.tensor.matmul(
                            s0_ps[:], ones_col[:],
                            xt[:, j * 512:(j + 1) * 512],
                            start=(c == 0 and j == 0),
                            stop=(c == NCHUNK - 1 and j == CH // 512 - 1))
                s0all = p_sm.tile([1, 512], F32, tag="s0all")
                nc.vector.tensor_copy(s0all[:], s0_ps[:])
                f1 = p_sm.tile([1, D], F32, tag="f1")
                TT(f1[:], s0all[:, 0:128], s0all[:, 128:256], OP.add)
                f2 = p_sm.tile([1, D], F32, tag="f2")
                TT(f2[:], s0all[:, 256:384], s0all[:, 384:512], OP.add)
                s0row = p_sm.tile([1, D], F32, tag="s0row")
                TT(s0row[:], f1[:], f2[:], OP.add)
                scol1 = row_to_cols(s0row, 1)
                C1_sb, nb32_1 = att_params(scol1, [0, 0])

                # ---------- pass B: attention round 1 ----------
                attbuf = p_att.tile([128, NCHUNK * NT * NH], F32, tag="attb")
                s1_ps = pp_acc.tile([NH, D], F32, tag="sacc")
                for c in range(NCHUNK):
                    xt = p_x.tile([128, CH], F32, tag="xt")
                    load_chunk(g, c, xt)
                    d_ps = pp_d.tile([128, NT * NH], F32, tag="d")
                    dots(xt, C1_sb, d_ps)
                    att_sl = attbuf[:, c * NT * NH:(c + 1) * NT * NH]
                    msk = p_sm.tile([128, NT * NH], F32, tag="msk")
                    TT(msk[:], d_ps[:], nb32_1[:], OP.is_gt)
                    TT(msk[:], msk[:], hm32_sb[:], OP.mult)
                    TT(att_sl, msk[:], lo32_sb[:], OP.add)
                    for t in range(NT):
                        nc.tensor.matmul(
                            s1_ps[:],
                            attbuf[:, (c * NT + t) * NH:(c * NT + t + 1) * NH],
                            xt[:, t * 128:(t + 1) * 128],
                            start=(c == 0 and t == 0),
                            stop=(c == NCHUNK - 1 and t == NT - 1))
                s1row = p_sm.tile([NH, D], F32, tag="s1row")
                nc.scalar.copy(s1row[:], s1_ps[:])
                scol2 = row_to_cols(s1row, NH)
                C2_sb, nb32_2 = att_params(scol2, [0, 1])

                # ---------- pass C: attention round 2 ----------
                s2_ps = pp_acc.tile([NH, D], F32, tag="sacc")
                for c in range(NCHUNK):
                    xt = p_x.tile([128, CH], F32, tag="xt")
                    load_chunk(g, c, xt)
                    d_ps = pp_d.tile([128, NT * NH], F32, tag="d")
                    dots(xt, C2_sb, d_ps)
                    att_sl = attbuf[:, c * NT * NH:(c + 1) * NT * NH]
                    tmp = p_sm.tile([128, NT * NH], F32, tag="tmp")
                    TT(tmp[:], d_ps[:], att_sl, OP.mult)
                    TT(tmp[:], tmp[:], nb32_2[:], OP.is_gt)
                    TT(tmp[:], tmp[:], hm32_sb[:], OP.mult)
                    TT(tmp[:], tmp[:], lo32_sb[:], OP.add)
                    w_sl = p_sm.tile([128, NT * NH], F32, tag="w")
                    TT(w_sl[:], tmp[:], att_sl, OP.mult)
                    for t in range(NT):
                        nc.tensor.matmul(
                            s2_ps[:], w_sl[:, t * NH:(t + 1) * NH],
                            xt[:, t * 128:(t + 1) * 128],
                            start=(c == 0 and t == 0),
                            stop=(c == NCHUNK - 1 and t == NT - 1))
                s2_sb = p_sm.tile([NH, D], F32, tag="s2sb")
                nc.scalar.copy(s2_sb[:], s2_ps[:])
                nc.sync.dma_start(out=out_ext[g], in_=s2_sb[:])

    nc.finalize()
    return nc


def _prep_shared(W_att, V_att, Wt_att, U_att, b_att):
    sig1 = np.float32(1.0 / (1.0 + np.exp(-1.0)))
    sigm1 = np.float32(1.0 / (1.0 + np.exp(1.0)))
    # wn[d, i*D+j] = W_att[i, d, j]/N  (lhsT layout: k=d, m=j per head)
    wn = np.ascontiguousarray(
        np.transpose(W_att / np.float32(N), (1, 0, 2)).reshape(D, NH * D)
    ).astype(np.float32)
    # wtt[e, i*D+d2] = Wt_att[i, 0, d2, e]  (lhsT layout: k=e, m=d2 per head)
    wtt = np.ascontiguousarray(
        np.transpose(Wt_att[:, 0, :, :], (2, 0, 1)).reshape(D, NH * D)
    ).astype(np.float32)
    vat = np.ascontiguousarray(V_att[:, 0, :D].T).astype(np.float32)   # (D, NH)
    vbt = np.ascontiguousarray(V_att[:, 0, D:].T).astype(np.float32)   # (D, NH)
    negb = np.tile((-b_att[:, 0]).astype(np.float32)[None, :], (D, 1))
    u = U_att[:, 0, 0].astype(np.float32)                    # (NH,)
    lo = u * sigm1                                           # (NH,)
    hm = u * sig1 - lo                                       # (NH,)
    lo32 = np.tile(lo[None, :], (D, NT)).astype(np.float32)  # (D, NT*NH)
    hm32 = np.tile(hm[None, :], (D, NT)).astype(np.float32)
    ident = np.eye(D, dtype=np.float32)
    return dict(wn=wn, wtt=wtt, vat=vat, vbt=vbt, negb=negb,
                lo32=lo32, hm32=hm32, ident=ident)


def _ntn_head(g1, g2, V_ntn, W_ntn, b_ntn, proj0, proj1, proj2, proj3):
    DIN2 = D * NH
    Va, Vb = V_ntn[:, :DIN2], V_ntn[:, DIN2:]
    s = Va @ g1 + Vb @ g2 + np.einsum("fde,d,e->f", W_ntn, g1, g2) + b_ntn
    s = s / max(np.sum(np.abs(s)), EPS)
    s = np.maximum(s, np.float32(0.0))
    y = proj3 @ (proj2 @ (proj1 @ (proj0 @ s)))
    return y.astype(np.float32)


def kernel(x1, x2, W_att, V_att, Wt_att, U_att, b_att,
           V_ntn, W_ntn, b_ntn, proj0, proj1, proj2, proj3):
    x1 = np.asarray(x1, dtype=np.float32)
    x2 = np.asarray(x2, dtype=np.float32)
    if "nc" not in _CACHED:
        _CACHED["nc"] = _build_nc()
    nc = _CACHED["nc"]
    shared = _prep_shared(np.asarray(W_att), np.asarray(V_att),
                          np.asarray(Wt_att), np.asarray(U_att),
                          np.asarray(b_att))
    in_maps = []
    for b in range(B):
        m = {"x1": np.ascontiguousarray(x1[b]),
             "x2": np.ascontiguousarray(x2[b])}
        m.update(shared)
        in_maps.append(m)
    res = run_bass_kernel_spmd(nc, in_maps, list(range(B)))
    V_ntn = np.asarray(V_ntn, dtype=np.float32)
    W_ntn = np.asarray(W_ntn, dtype=np.float32)
    b_ntn = np.asarray(b_ntn, dtype=np.float32)
    projs = [np.asarray(p, dtype=np.float32) for p in (proj0, proj1, proj2, proj3)]
    out = np.zeros((B, 1), dtype=np.float32)
    for b in range(B):
        g = res.results[b]["out"]          # (2, NH, D)
        g1 = g[0].reshape(NH * D)
        g2 = g[1].reshape(NH * D)
        out[b] = _ntn_head(g1, g2, V_ntn, W_ntn, b_ntn, *projs)
    return out
